# revision 1
# baseline (speedup 1.0000x reference)
"""Locally-connected (masked linear) layer for 8 TRN2 NeuronCores.

y = x @ (W * M)^T + b
  x: [4096, 4096] f32, W/M: [4096, 4096] f32, b: [4096] f32.

Strategy (tensor-parallel over out_features):
  - Each core owns a 512-row shard of W/M (and of the output columns).
  - The contraction dim is the minor dim of both x and W in HBM, so the
    host uploads x^T, W^T, M^T (contraction-major) in bf16; the device
    never needs a transpose.
  - Device: masked_w = W^T * M^T (DVE, bf16, exact since M is 0/1),
    then PE matmuls accumulate y^T = masked_w^T.T @ x^T in fp32 PSUM,
    bias is added per-partition on evacuation, y^T shard DMAs out fp32.
  - Host concatenates the 8 y^T shards and transposes back.
"""

import os

import numpy as np
import ml_dtypes

BATCH = 4096
IN_F = 4096
OUT_F = 4096
N_CORES = 8
O_SHARD = OUT_F // N_CORES  # 512
P = 128                     # SBUF partitions
BG = 512                    # batch columns per PSUM accumulation group
XCH = 4                     # k-tiles per x DMA slab

_BF16 = ml_dtypes.bfloat16
_NC = None
LAST_RESULT = None


def _ensure_axon_hooks_stub():
    """bass_utils' axon trace path imports antenv.axon_hooks, which this
    container's antenv stub lacks. Install a minimal registry so the
    import succeeds (hook None => bass_utils skips tracing gracefully)."""
    import sys
    import types

    try:
        import antenv.axon_hooks  # noqa: F401
        return
    except ImportError:
        pass
    import antenv

    mod = types.ModuleType("antenv.axon_hooks")
    mod._HOOK = None

    def set_axon_ntff_profile_hook(h):
        mod._HOOK = h

    def get_axon_ntff_profile_hook():
        return mod._HOOK

    mod.set_axon_ntff_profile_hook = set_axon_ntff_profile_hook
    mod.get_axon_ntff_profile_hook = get_axon_ntff_profile_hook
    antenv.axon_hooks = mod
    sys.modules["antenv.axon_hooks"] = mod


def _install_real_ntff_hook():
    """Wire the ctypes NTFF profiling hook (normally registered by the
    boot middleware) so run_bass_kernel_spmd(trace=True) works."""
    _ensure_axon_hooks_stub()
    import antenv.axon_hooks as ah

    if ah.get_axon_ntff_profile_hook() is None:
        try:
            from trn_agent_boot.trn_boot import _ntff_profile_via_ctypes

            hook = _ntff_profile_via_ctypes("/opt/axon/libaxon_pjrt.so")
            if hook is not None:
                ah.set_axon_ntff_profile_hook(hook)
        except Exception:
            pass
    try:
        import concourse.bass_utils as bu

        bu.upload_artifacts = lambda tmpdir: "local://" + str(tmpdir)
    except Exception:
        pass


def build_nc(batch=BATCH, in_f=IN_F, o_shard=O_SHARD, bg=BG, xch=XCH):
    import concourse.mybir as mybir
    from concourse import bacc
    from concourse.tile import TileContext

    p = P
    kt = in_f // p          # k tiles along contraction
    oc = o_shard // p       # out-feature chunks of 128
    ng = batch // bg        # batch groups
    bf16 = mybir.dt.bfloat16
    f32 = mybir.dt.float32

    nc = bacc.Bacc()
    xT = nc.declare_dram_parameter("xT", [in_f, batch], bf16, isOutput=False)
    wT = nc.declare_dram_parameter("wT", [in_f, o_shard], bf16, isOutput=False)
    mT = nc.declare_dram_parameter("mT", [in_f, o_shard], bf16, isOutput=False)
    bT = nc.declare_dram_parameter("bT", [p, oc], f32, isOutput=False)
    yT = nc.declare_dram_parameter("yT", [o_shard, batch], f32, isOutput=True)

    xv = xT[:].rearrange("(c p) b -> p c b", p=p)   # [128, kt, batch]
    wv = wT[:].rearrange("(c p) o -> p c o", p=p)   # [128, kt, o_shard]
    mv = mT[:].rearrange("(c p) o -> p c o", p=p)

    with TileContext(nc) as tc:
        with tc.tile_pool(name="const", bufs=1) as cpool, \
             tc.tile_pool(name="stage", bufs=4) as spool, \
             tc.tile_pool(name="xin", bufs=6) as xpool, \
             tc.tile_pool(name="acc", bufs=8, space="PSUM") as ppool, \
             tc.tile_pool(name="out", bufs=4) as opool:

            bias_t = cpool.tile([p, oc], f32)
            nc.sync.dma_start(out=bias_t, in_=bT[:])

            # masked weights, resident in SBUF for the whole kernel
            mw = cpool.tile([p, kt, o_shard], bf16)
            for k in range(kt):
                wst = spool.tile([p, o_shard], bf16, tag="w")
                mst = spool.tile([p, o_shard], bf16, tag="m")
                nc.sync.dma_start(out=wst, in_=wv[:, k, :])
                nc.sync.dma_start(out=mst, in_=mv[:, k, :])
                nc.vector.tensor_mul(out=mw[:, k, :], in0=wst, in1=mst)

            for g in range(ng):
                psums = [ppool.tile([p, bg], f32, tag="ps", name=f"ps{g}_{j}")
                         for j in range(oc)]
                xt = None
                for k in range(kt):
                    if k % xch == 0:
                        xt = xpool.tile([p, xch, bg], bf16, tag="x")
                        nc.sync.dma_start(
                            out=xt,
                            in_=xv[:, k:k + xch, g * bg:(g + 1) * bg],
                        )
                    rhs = xt[:, k % xch, :]
                    for j in range(oc):
                        nc.tensor.matmul(
                            psums[j],
                            mw[:, k, j * p:(j + 1) * p],
                            rhs,
                            start=(k == 0),
                            stop=(k == kt - 1),
                        )
                for j in range(oc):
                    ot = opool.tile([p, bg], f32, tag="o")
                    nc.vector.tensor_scalar_add(
                        out=ot, in0=psums[j], scalar1=bias_t[:, j:j + 1]
                    )
                    nc.sync.dma_start(
                        out=yT[j * p:(j + 1) * p, g * bg:(g + 1) * bg],
                        in_=ot,
                    )
    nc.finalize()
    return nc


def _prep_in_maps(x, weight, bias, myFilter):
    oc = O_SHARD // P
    xTb = np.ascontiguousarray(np.asarray(x, np.float32).T).astype(_BF16)
    in_maps = []
    for c in range(N_CORES):
        rows = slice(c * O_SHARD, (c + 1) * O_SHARD)
        wTb = np.ascontiguousarray(
            np.asarray(weight, np.float32)[rows].T).astype(_BF16)
        mTb = np.ascontiguousarray(
            np.asarray(myFilter, np.float32)[rows].T).astype(_BF16)
        bTb = np.ascontiguousarray(
            np.asarray(bias, np.float32)[rows].reshape(oc, P).T)
        in_maps.append({"xT": xTb, "wT": wTb, "mT": mTb, "bT": bTb})
    return in_maps


def kernel(x, weight, bias, myFilter):
    global _NC, LAST_RESULT
    _ensure_axon_hooks_stub()
    from concourse.bass_utils import run_bass_kernel_spmd

    if _NC is None:
        _NC = build_nc()

    in_maps = _prep_in_maps(x, weight, bias, myFilter)

    kwargs = {}
    if os.environ.get("KERNEL_TRACE") == "1":
        _install_real_ntff_hook()
        kwargs["trace"] = True
        tdir = os.environ.get("KERNEL_TRACE_DIR")
        if tdir:
            kwargs["tmpdir"] = tdir

    res = run_bass_kernel_spmd(_NC, in_maps, list(range(N_CORES)), **kwargs)
    LAST_RESULT = res

    yT = np.concatenate(
        [res.results[c]["yT"] for c in range(N_CORES)], axis=0)
    return np.ascontiguousarray(yT.T)



# revision 3
# speedup vs baseline: 1.1633x; 1.1633x over previous
"""Locally-connected (masked linear) layer for 8 TRN2 NeuronCores.

y = x @ (W * M)^T + b
  x: [4096, 4096] f32, W/M: [4096, 4096] f32, b: [4096] f32.

Strategy (tensor-parallel over out_features):
  - Each core owns a 512-row shard of W/M (and of the output columns).
  - Host premultiplies mw = W * M (exact masking), uploads x^T and mw^T
    contraction-major in bf16 so the device never transposes anything.
  - Device: PE matmuls accumulate y^T = mw^T.T @ x^T in fp32 PSUM,
    bias is added per-partition on evacuation, y^T shard DMAs out bf16.
  - DMA rings: x slabs stream on the sync (SP) HWDGE ring; weights, bias
    and outputs ride the scalar (Activation) ring so the x stream is
    never queued behind the 4MB weight upload (FIFO per ring).
  - The first pass interleaves batch groups 0+1 (all 8 PSUM banks) so
    the PE has 2x work per arriving weight tile while weights stream in;
    later groups run singly off the SBUF-resident weights.
  - Host concatenates the 8 y^T shards, upcasts and transposes back.
"""

import os

import numpy as np
import ml_dtypes

BATCH = 4096
IN_F = 4096
OUT_F = 4096
N_CORES = 8
O_SHARD = OUT_F // N_CORES  # 512
P = 128                     # SBUF partitions
BG = 512                    # batch columns per PSUM accumulation group
XCH = 4                     # k-tiles per x DMA slab

_BF16 = ml_dtypes.bfloat16
_NC = None
LAST_RESULT = None


def _ensure_axon_hooks_stub():
    """bass_utils' axon trace path imports antenv.axon_hooks, which this
    container's antenv stub lacks. Install a minimal registry so the
    import succeeds (hook None => bass_utils skips tracing gracefully)."""
    import sys
    import types

    try:
        import antenv.axon_hooks  # noqa: F401
        return
    except ImportError:
        pass
    import antenv

    mod = types.ModuleType("antenv.axon_hooks")
    mod._HOOK = None

    def set_axon_ntff_profile_hook(h):
        mod._HOOK = h

    def get_axon_ntff_profile_hook():
        return mod._HOOK

    mod.set_axon_ntff_profile_hook = set_axon_ntff_profile_hook
    mod.get_axon_ntff_profile_hook = get_axon_ntff_profile_hook
    antenv.axon_hooks = mod
    sys.modules["antenv.axon_hooks"] = mod


def _install_real_ntff_hook():
    """Wire the ctypes NTFF profiling hook (normally registered by the
    boot middleware) so run_bass_kernel_spmd(trace=True) works."""
    _ensure_axon_hooks_stub()
    import antenv.axon_hooks as ah

    if ah.get_axon_ntff_profile_hook() is None:
        try:
            from trn_agent_boot.trn_boot import _ntff_profile_via_ctypes

            hook = _ntff_profile_via_ctypes("/opt/axon/libaxon_pjrt.so")
            if hook is not None:
                ah.set_axon_ntff_profile_hook(hook)
        except Exception:
            pass
    try:
        import concourse.bass_utils as bu

        bu.upload_artifacts = lambda tmpdir: "local://" + str(tmpdir)
    except Exception:
        pass


def build_nc(batch=BATCH, in_f=IN_F, o_shard=O_SHARD, bg=BG, xch=XCH):
    import concourse.mybir as mybir
    from concourse import bacc
    from concourse.tile import TileContext

    p = P
    kt = in_f // p          # k tiles along contraction
    oc = o_shard // p       # out-feature chunks of 128
    ng = batch // bg        # batch groups
    bf16 = mybir.dt.bfloat16
    f32 = mybir.dt.float32

    nc = bacc.Bacc()
    xT = nc.declare_dram_parameter("xT", [in_f, batch], bf16, isOutput=False)
    mwT = nc.declare_dram_parameter("mwT", [in_f, o_shard], bf16,
                                    isOutput=False)
    bT = nc.declare_dram_parameter("bT", [p, oc], f32, isOutput=False)
    yT = nc.declare_dram_parameter("yT", [o_shard, batch], bf16,
                                   isOutput=True)

    xv = xT[:].rearrange("(c p) b -> p c b", p=p)   # [128, kt, batch]
    wv = mwT[:].rearrange("(c p) o -> p c o", p=p)  # [128, kt, o_shard]

    # batch-group schedule: the first pass runs two groups interleaved so
    # the PE has 2x work per k-tile while the weights are still streaming
    # in; all later groups run singly off the resident weights.
    if ng >= 2:
        phases = [[0, 1]] + [[g] for g in range(2, ng)]
    else:
        phases = [[0]]

    with TileContext(nc) as tc:
        with tc.tile_pool(name="const", bufs=1) as cpool, \
             tc.tile_pool(name="xin", bufs=6) as xpool, \
             tc.tile_pool(name="acc", bufs=8, space="PSUM") as ppool, \
             tc.tile_pool(name="out", bufs=4) as opool:

            bias_t = cpool.tile([p, oc], f32)
            nc.scalar.dma_start(out=bias_t, in_=bT[:])

            # masked weights, resident in SBUF for the whole kernel;
            # per-k-tile DMAs on the scalar ring pace with consumption
            mw = cpool.tile([p, kt, o_shard], bf16)
            for k in range(kt):
                nc.scalar.dma_start(out=mw[:, k, :], in_=wv[:, k, :])

            for groups in phases:
                psums = {}
                for g in groups:
                    for j in range(oc):
                        psums[(g, j)] = ppool.tile(
                            [p, bg], f32, tag="ps", name=f"ps{g}_{j}")
                xts = {}
                for k in range(kt):
                    if k % xch == 0:
                        for g in groups:
                            xts[g] = xpool.tile([p, xch, bg], bf16, tag="x",
                                                name=f"x{g}_{k}")
                            nc.sync.dma_start(
                                out=xts[g],
                                in_=xv[:, k:k + xch, g * bg:(g + 1) * bg],
                            )
                    for j in range(oc):
                        for g in groups:
                            nc.tensor.matmul(
                                psums[(g, j)],
                                mw[:, k, j * p:(j + 1) * p],
                                xts[g][:, k % xch, :],
                                start=(k == 0),
                                stop=(k == kt - 1),
                            )
                for g in groups:
                    for j in range(oc):
                        ot = opool.tile([p, bg], bf16, tag="o")
                        nc.vector.tensor_scalar_add(
                            out=ot, in0=psums[(g, j)],
                            scalar1=bias_t[:, j:j + 1],
                        )
                        nc.scalar.dma_start(
                            out=yT[j * p:(j + 1) * p, g * bg:(g + 1) * bg],
                            in_=ot,
                        )
    nc.finalize()
    return nc


def _prep_in_maps(x, weight, bias, myFilter):
    oc = O_SHARD // P
    xTb = np.ascontiguousarray(np.asarray(x, np.float32).T).astype(_BF16)
    mw = np.asarray(weight, np.float32) * np.asarray(myFilter, np.float32)
    in_maps = []
    for c in range(N_CORES):
        rows = slice(c * O_SHARD, (c + 1) * O_SHARD)
        mwTb = np.ascontiguousarray(mw[rows].T).astype(_BF16)
        bTb = np.ascontiguousarray(
            np.asarray(bias, np.float32)[rows].reshape(oc, P).T)
        in_maps.append({"xT": xTb, "mwT": mwTb, "bT": bTb})
    return in_maps


def kernel(x, weight, bias, myFilter):
    global _NC, LAST_RESULT
    _ensure_axon_hooks_stub()
    from concourse.bass_utils import run_bass_kernel_spmd

    if _NC is None:
        _NC = build_nc()

    in_maps = _prep_in_maps(x, weight, bias, myFilter)

    kwargs = {}
    if os.environ.get("KERNEL_TRACE") == "1":
        _install_real_ntff_hook()
        kwargs["trace"] = True
        tdir = os.environ.get("KERNEL_TRACE_DIR")
        if tdir:
            kwargs["tmpdir"] = tdir

    res = run_bass_kernel_spmd(_NC, in_maps, list(range(N_CORES)), **kwargs)
    LAST_RESULT = res

    yT = np.concatenate(
        [np.asarray(res.results[c]["yT"]) for c in range(N_CORES)], axis=0)
    return np.ascontiguousarray(yT.T.astype(np.float32))


# revision 5
# speedup vs baseline: 1.1675x; 1.0035x over previous
"""Locally-connected (masked linear) layer for 8 TRN2 NeuronCores.

y = x @ (W * M)^T + b
  x: [4096, 4096] f32, W/M: [4096, 4096] f32, b: [4096] f32.

Strategy (tensor-parallel over out_features):
  - Each core owns a 512-row shard of W/M (and of the output columns).
  - Host premultiplies mw = W * M (exact masking), uploads x^T and mw^T
    contraction-major in bf16 so the device never transposes anything.
  - Device: PE matmuls accumulate y^T = mw^T.T @ x^T in fp32 PSUM,
    bias is added per-partition on evacuation, y^T shard DMAs out bf16.
  - DMA rings: x slabs stream on the sync (SP) HWDGE ring; weights, bias
    and outputs ride the scalar (Activation) ring so the x stream is
    never queued behind the 4MB weight upload (FIFO per ring).
  - The first pass interleaves batch groups 0+1 (all 8 PSUM banks) so
    the PE has 2x work per arriving weight tile while weights stream in;
    later groups run singly off the SBUF-resident weights.
  - Host concatenates the 8 y^T shards, upcasts and transposes back.
"""

import os

import numpy as np
import ml_dtypes

BATCH = 4096
IN_F = 4096
OUT_F = 4096
N_CORES = 8
O_SHARD = OUT_F // N_CORES  # 512
P = 128                     # SBUF partitions
BG = 512                    # batch columns per PSUM accumulation group
XCH = 4                     # k-tiles per x DMA slab

_BF16 = ml_dtypes.bfloat16
_NC = None
LAST_RESULT = None


def _ensure_axon_hooks_stub():
    """bass_utils' axon trace path imports antenv.axon_hooks, which this
    container's antenv stub lacks. Install a minimal registry so the
    import succeeds (hook None => bass_utils skips tracing gracefully)."""
    import sys
    import types

    try:
        import antenv.axon_hooks  # noqa: F401
        return
    except ImportError:
        pass
    import antenv

    mod = types.ModuleType("antenv.axon_hooks")
    mod._HOOK = None

    def set_axon_ntff_profile_hook(h):
        mod._HOOK = h

    def get_axon_ntff_profile_hook():
        return mod._HOOK

    mod.set_axon_ntff_profile_hook = set_axon_ntff_profile_hook
    mod.get_axon_ntff_profile_hook = get_axon_ntff_profile_hook
    antenv.axon_hooks = mod
    sys.modules["antenv.axon_hooks"] = mod


def _install_real_ntff_hook():
    """Wire the ctypes NTFF profiling hook (normally registered by the
    boot middleware) so run_bass_kernel_spmd(trace=True) works."""
    _ensure_axon_hooks_stub()
    import antenv.axon_hooks as ah

    if ah.get_axon_ntff_profile_hook() is None:
        try:
            from trn_agent_boot.trn_boot import _ntff_profile_via_ctypes

            hook = _ntff_profile_via_ctypes("/opt/axon/libaxon_pjrt.so")
            if hook is not None:
                ah.set_axon_ntff_profile_hook(hook)
        except Exception:
            pass
    try:
        import concourse.bass_utils as bu

        bu.upload_artifacts = lambda tmpdir: "local://" + str(tmpdir)
    except Exception:
        pass


def build_nc(batch=BATCH, in_f=IN_F, o_shard=O_SHARD, bg=BG, xch=XCH):
    import concourse.mybir as mybir
    from concourse import bacc
    from concourse.tile import TileContext

    p = P
    kt = in_f // p          # k tiles along contraction
    oc = o_shard // p       # out-feature chunks of 128
    ng = batch // bg        # batch groups
    bf16 = mybir.dt.bfloat16
    f32 = mybir.dt.float32

    nc = bacc.Bacc()
    xT = nc.declare_dram_parameter("xT", [in_f, batch], bf16, isOutput=False)
    mwT = nc.declare_dram_parameter("mwT", [in_f, o_shard], bf16,
                                    isOutput=False)
    bT = nc.declare_dram_parameter("bT", [p, oc], f32, isOutput=False)
    yT = nc.declare_dram_parameter("yT", [o_shard, batch], bf16,
                                   isOutput=True)

    xv = xT[:].rearrange("(c p) b -> p c b", p=p)   # [128, kt, batch]
    wv = mwT[:].rearrange("(c p) o -> p c o", p=p)  # [128, kt, o_shard]

    # batch groups run in interleaved pairs: one x slab / out tile spans
    # both groups (2KB DMA lines halve the descriptor count), and the PE
    # has 2x work per k-tile while the weights stream in at the start.
    assert ng % 2 == 0
    npair = ng // 2
    bg2 = 2 * bg

    with TileContext(nc) as tc:
        with tc.tile_pool(name="const", bufs=1) as cpool, \
             tc.tile_pool(name="xin", bufs=6) as xpool, \
             tc.tile_pool(name="acc", bufs=8, space="PSUM") as ppool, \
             tc.tile_pool(name="out", bufs=4) as opool:

            bias_t = cpool.tile([p, oc], f32)
            nc.scalar.dma_start(out=bias_t, in_=bT[:])

            # masked weights, resident in SBUF for the whole kernel;
            # per-k-tile DMAs on the scalar ring pace with consumption
            mw = cpool.tile([p, kt, o_shard], bf16)
            for k in range(kt):
                nc.scalar.dma_start(out=mw[:, k, :], in_=wv[:, k, :])

            for pair in range(npair):
                cols = slice(pair * bg2, (pair + 1) * bg2)
                psums = {}
                for h in range(2):
                    for j in range(oc):
                        psums[(h, j)] = ppool.tile(
                            [p, bg], f32, tag="ps", name=f"ps{pair}_{h}_{j}")
                xt = None
                for k in range(kt):
                    if k % xch == 0:
                        xt = xpool.tile([p, xch, bg2], bf16, tag="x",
                                        name=f"x{pair}_{k}")
                        if pair == 0 and k == 0:
                            # split the very first slab so k-tile 0 lands
                            # alone and the first matmuls fire early
                            nc.sync.dma_start(
                                out=xt[:, 0:1, :], in_=xv[:, 0:1, cols])
                            nc.sync.dma_start(
                                out=xt[:, 1:xch, :], in_=xv[:, 1:xch, cols])
                        else:
                            nc.sync.dma_start(
                                out=xt, in_=xv[:, k:k + xch, cols])
                    for j in range(oc):
                        for h in range(2):
                            nc.tensor.matmul(
                                psums[(h, j)],
                                mw[:, k, j * p:(j + 1) * p],
                                xt[:, k % xch, h * bg:(h + 1) * bg],
                                start=(k == 0),
                                stop=(k == kt - 1),
                            )
                for j in range(oc):
                    # evacuations split across DVE and ACT so two engines
                    # drain PSUM banks in parallel (GpSimd can't read PSUM)
                    ot = opool.tile([p, bg2], bf16, tag="o",
                                    name=f"o{pair}_{j}")
                    for h in range(2):
                        if j % 2 == 0:
                            nc.vector.tensor_scalar_add(
                                out=ot[:, h * bg:(h + 1) * bg],
                                in0=psums[(h, j)],
                                scalar1=bias_t[:, j:j + 1],
                            )
                        else:
                            nc.scalar.add(
                                ot[:, h * bg:(h + 1) * bg],
                                psums[(h, j)],
                                bias_t[:, j:j + 1],
                            )
                    nc.scalar.dma_start(
                        out=yT[j * p:(j + 1) * p, cols], in_=ot)
    nc.finalize()
    return nc


def _prep_in_maps(x, weight, bias, myFilter):
    oc = O_SHARD // P
    xTb = np.ascontiguousarray(np.asarray(x, np.float32).T).astype(_BF16)
    mw = np.asarray(weight, np.float32) * np.asarray(myFilter, np.float32)
    in_maps = []
    for c in range(N_CORES):
        rows = slice(c * O_SHARD, (c + 1) * O_SHARD)
        mwTb = np.ascontiguousarray(mw[rows].T).astype(_BF16)
        bTb = np.ascontiguousarray(
            np.asarray(bias, np.float32)[rows].reshape(oc, P).T)
        in_maps.append({"xT": xTb, "mwT": mwTb, "bT": bTb})
    return in_maps


def kernel(x, weight, bias, myFilter):
    global _NC, LAST_RESULT
    _ensure_axon_hooks_stub()
    from concourse.bass_utils import run_bass_kernel_spmd

    if _NC is None:
        _NC = build_nc()

    in_maps = _prep_in_maps(x, weight, bias, myFilter)

    kwargs = {}
    if os.environ.get("KERNEL_TRACE") == "1":
        _install_real_ntff_hook()
        kwargs["trace"] = True
        tdir = os.environ.get("KERNEL_TRACE_DIR")
        if tdir:
            kwargs["tmpdir"] = tdir

    res = run_bass_kernel_spmd(_NC, in_maps, list(range(N_CORES)), **kwargs)
    LAST_RESULT = res

    yT = np.concatenate(
        [np.asarray(res.results[c]["yT"]) for c in range(N_CORES)], axis=0)
    return np.ascontiguousarray(yT.T.astype(np.float32))
